# revision 5
# baseline (speedup 1.0000x reference)
"""Trainium2 Bass kernel for repeated sparse COO SpMM (GNN message passing).

y <- A @ y applied LAYERS times, A[row[e], col[e]] = weights[e].
N=100000 nodes, E=3200000 edges, B=16 features, 4 layers, 8 NeuronCores.

v4 strategy (1D partition by destination row):
  * Host: relabel nodes into 16 degree-snaked shards (core c owns shards
    c and c+8); per destination-tile (128 dests) the on-chip segment-sum
    is one strided DVE reduce with D = max degree in tile (degree-sorted
    tiles make the padding ~1%).
  * y is f32, packed 4 nodes per 256B row (dma_gather's stride quantum),
    so the whole graph fits ONE int16 gather window (25088 blocks). Each
    edge gathers the 256B block holding its source node; a host-built
    weight mask selects the node during the DVE multiply, and the
    strided reduce sums over D*4 sub-slots in f32.
  * dma_gather descriptor generation runs on one Q7 core pair per SWDGE
    queue; calls round-robin over all 4 queues so all 8 Q7 cores
    generate descriptors in parallel.
  * Each layer's AllGather is split in two (shards 0-7, then 8-15): the
    first fires mid-layer and overlaps the second half's gathers. The
    AllGather output IS the packed layout (same bytes), so the next
    layer gathers straight from it.
"""

import numpy as np

# ---------------------------------------------------------------- problem dims
N_NODES = 100000
N_EDGES = 3200000
BATCH = 16
LAYERS = 4
NCORES = 8
NSHARDS = 16
P = 128
PACK = 4  # nodes per 256B gather block (f32)
ROWF = PACK * BATCH  # 64 f32 per packed row

SLOT_BUDGET = 128  # msg-buffer slots per chunk (x256B per partition)
NUM_QUEUES = 4
CALL_SLOTS = 8  # slot-columns per dma_gather call (x128 = 1024 idxs)


def _mk_chunks(dt, t0, t1):
    chunks = []
    t = t0
    while t < t1:
        te = t
        acc = 0
        while te < t1 and (te == t or acc + dt[te] <= SLOT_BUDGET):
            acc += dt[te]
            te += 1
        chunks.append((t, te))
        t = te
    return chunks


class _Prep:
    """Host-side graph preprocessing, shared by kernel() and tests."""

    def __init__(self, x, weights, row, col, n_nodes, ncores, layers):
        n = n_nodes
        nps_real = n // NSHARDS  # real nodes per shard
        tiles_s = (nps_real + P - 1) // P  # tiles per shard
        nps = tiles_s * P  # padded shard size
        tiles = 2 * tiles_s  # tiles per core
        npc = 2 * nps  # positions per core
        npad = NSHARDS * nps
        nblocks = npad // PACK
        assert nblocks < 32768  # one int16 gather window

        row = np.asarray(row).astype(np.int64)
        col = np.asarray(col).astype(np.int64)
        weights = np.asarray(weights, dtype=np.float32)
        deg = np.bincount(row, minlength=n)

        # ascending-degree order, snake-assigned to 16 shards
        order = np.argsort(deg, kind="stable")
        blocks = order.reshape(nps_real, NSHARDS).copy()
        blocks[1::2] = blocks[1::2, ::-1]
        perm = np.empty(n, dtype=np.int64)
        for s in range(NSHARDS):
            perm[blocks[:, s]] = s * nps + np.arange(nps_real)

        new_row = perm[row]
        new_col = perm[col]

        # global position -> (core, local dest index)
        def to_core_local(p):
            v = p // nps
            return v % ncores, (v // ncores) * nps + p % nps

        # --- per-edge slot within its destination bucket -----------------
        eorder = np.argsort(new_row, kind="stable")
        sr = new_row[eorder]
        sc = new_col[eorder]
        sw_weights = weights[eorder]
        change = np.flatnonzero(np.diff(sr)) + 1
        starts = np.concatenate(([0], change))
        counts = np.diff(np.concatenate((starts, [len(sr)])))
        dests = sr[starts]
        j = np.arange(len(sr)) - np.repeat(starts, counts)  # rank in bucket

        dest_core, dest_local = to_core_local(dests)
        dest_ltile = dest_local // P
        e_core = np.repeat(dest_core, counts)
        e_ltile = np.repeat(dest_ltile, counts)
        e_p = np.repeat(dest_local, counts) % P

        dtc = np.ones((ncores, tiles), dtype=np.int64)
        np.maximum.at(dtc, (dest_core, dest_ltile), counts)
        # all cores share one program => tile widths must match across cores
        dt = dtc.max(axis=0)  # [tiles]
        off = np.concatenate(([0], np.cumsum(dt)[:-1]))  # col base per tile
        total_cols = int(dt.sum())

        # chunks per half (never straddle the shard boundary tile)
        chunks_a = _mk_chunks(dt, 0, tiles_s)
        chunks_b = _mk_chunks(dt, tiles_s, tiles)
        chunks = chunks_a + chunks_b
        chunk_cols = np.array([int(dt[a:b].sum()) for a, b in chunks])
        chunk_col_base = np.concatenate(([0], np.cumsum(chunk_cols)[:-1]))

        # --- per-edge slot column, weight mask, idx table ----------------
        e_slot = off[e_ltile] + j  # global slot column
        blk = sc // PACK
        sub = sc % PACK

        w8 = np.zeros((ncores, P, total_cols * PACK), dtype=np.float32)
        w8[e_core, e_p, e_slot * PACK + sub] = sw_weights

        # gather entry j -> partition j%128, free j//128 (slot col)
        e_entry = e_slot * P + e_p
        total_entries = total_cols * P
        flat_idx = np.zeros((ncores, total_entries), dtype=np.int16)
        flat_idx[e_core, e_entry] = blk.astype(np.int16)
        assert total_entries % 16 == 0
        wrapped = flat_idx.reshape(ncores, total_entries // 16, 16).transpose(
            0, 2, 1
        )
        idx16_all = np.ascontiguousarray(np.tile(wrapped, (1, 8, 1)))

        xp = np.zeros((npad, BATCH), dtype=np.float32)
        xp[perm] = np.asarray(x, dtype=np.float32)
        xpad = np.ascontiguousarray(xp.reshape(nblocks, ROWF))

        # unshard: node n -> y_concat[core*npc + local]
        pc, pl = to_core_local(perm)
        self.unshard = pc * npc + pl

        self.n_nodes = n
        self.ncores = ncores
        self.layers = layers
        self.tiles = tiles
        self.tiles_s = tiles_s
        self.nps = nps
        self.npc = npc
        self.npad = npad
        self.nblocks = nblocks
        self.dt = dt
        self.off = off
        self.chunks = chunks
        self.n_chunks_a = len(chunks_a)
        self.chunk_cols = chunk_cols
        self.chunk_col_base = chunk_col_base
        self.total_cols = total_cols
        self.total_entries = total_entries
        self.perm = perm
        self.w8 = w8  # f32 weight mask
        self.idx16_all = idx16_all
        self.xpad = xpad
        self.slots = int(total_cols) * P


_REG_CACHE = {}


def _dma_gather_raw(
    g, out_ap, in_ap, idxs_ap, num_idxs, elem_size, elem_step, queue_num
):
    """Non-transpose DRAM-source dma_gather without the (transpose-only)
    elem_size%256 restriction. Mirrors bass.BassGpSimd.dma_gather."""
    import concourse.mybir as mybir

    stride_bytes = elem_step * mybir.dt.size(in_ap.dtype)
    assert stride_bytes % 256 == 0
    _in_ap = g.lower_ap_dma(in_ap, for_custom_bir_dma=True)
    _idxs_ap = g.lower_ap(idxs_ap)
    _out_ap = g.lower_ap(out_ap)
    key = (id(g), num_idxs)
    if key not in _REG_CACHE:
        _REG_CACHE[key] = g.to_reg(num_idxs)
    return g.add_instruction(
        mybir.InstDMAGatherAnt(
            name=g.bass.get_next_instruction_name(),
            ins=[*_in_ap, _idxs_ap, g.lower_val_access(_REG_CACHE[key])],
            outs=[_out_ap],
            transpose=False,
            num_idxs=num_idxs,
            elem_size=elem_size,
            stride_bytes_256=stride_bytes // 256,
            gen_mode=0,
            single_packet=True,
            queue_num=queue_num,
            sbuf_tokens_per_rank=0,
            sbuf_free_dim_per_rank=0,
            sbuf_free_dim_pad_per_rank=0,
            sbuf_byte_offset=0,
        )
    )


def build_program(prep):
    import concourse.bass as bass
    import concourse.bacc as bacc
    import concourse.mybir as mybir
    import concourse.tile as tile

    ncores = prep.ncores
    npc = prep.npc
    nps = prep.nps
    npad = prep.npad
    layers = prep.layers
    ts = prep.tiles_s

    nc = bacc.Bacc(
        None,
        num_devices=ncores,
        num_swdge_queues=NUM_QUEUES,
    )
    f32 = mybir.dt.float32
    i16 = mybir.dt.int16
    xfull = nc.dram_tensor(
        "xfull", [prep.nblocks, ROWF], f32, kind="ExternalInput"
    )
    idx_d = nc.dram_tensor(
        "idx", [P, prep.total_entries // 16], i16, kind="ExternalInput"
    )
    w_d = nc.dram_tensor(
        "w", [P, prep.total_cols * PACK], f32, kind="ExternalInput"
    )
    yout = nc.dram_tensor("yout", [npc, BATCH], f32, kind="ExternalOutput")

    with tile.TileContext(nc) as tc:
        with (
            tc.tile_pool(name="res", bufs=1) as res_pool,
            tc.tile_pool(name="msgp", bufs=3) as msg_pool,
            tc.tile_pool(name="idxp", bufs=12) as idx_pool,
            tc.tile_pool(name="outp", bufs=1) as out_pool,
            tc.tile_pool(name="dram", bufs=1, space="DRAM") as dram_pool,
        ):
            w_s = res_pool.tile([P, prep.total_cols * PACK], f32, name="w_s")
            nc.sync.dma_start(out=w_s[:], in_=w_d[:])
            # prime the DVE dependency on the w_s load
            w_prime = res_pool.tile([P, 1], f32, name="w_prime")
            nc.vector.tensor_copy(out=w_prime[:], in_=w_s[:, 0:1])

            slices = [
                dram_pool.tile([npc, BATCH], f32, name=f"slice{i}")
                for i in range(max(layers - 1, 1))
            ]
            ags = [
                [
                    dram_pool.tile(
                        [npad // 2, BATCH],
                        f32,
                        addr_space="Shared",
                        name=f"ag{i}_{h}",
                    )
                    for h in range(2)
                ]
                for i in range(max(layers - 1, 1))
            ]
            ybufs = [
                dram_pool.tile([npad, BATCH], f32, name=f"ybuf{i}")
                for i in range(max(layers - 1, 1))
            ]

            qn = 0
            for l in range(layers):
                if l == 0:
                    src = xfull[:, :]
                else:
                    src = ybufs[l - 1][:, :].rearrange(
                        "(n k) b -> n (k b)", k=PACK
                    )
                dst = yout if l == layers - 1 else slices[l]
                ylayer = out_pool.tile(
                    [P, prep.tiles * BATCH], f32, name=f"ylayer{l}", tag="yl"
                )

                def emit_half(h):
                    nonlocal qn
                    lo = 0 if h == 0 else prep.n_chunks_a
                    hi = prep.n_chunks_a if h == 0 else len(prep.chunks)
                    for ci in range(lo, hi):
                        t0, t1 = prep.chunks[ci]
                        ccols = int(prep.chunk_cols[ci])
                        cb = int(prep.chunk_col_base[ci])
                        msg = msg_pool.tile(
                            [P, ccols * ROWF], f32, name="msg", tag="msg"
                        )
                        for s0 in range(0, ccols, CALL_SLOTS):
                            sw = min(CALL_SLOTS, ccols - s0)
                            n_idx = P * sw
                            eb = (cb + s0) * P
                            idxt = idx_pool.tile(
                                [P, n_idx // 16], i16, name="idxt", tag="idxt"
                            )
                            nc.sync.dma_start(
                                out=idxt[:],
                                in_=idx_d[:, eb // 16 : (eb + n_idx) // 16],
                            )
                            _dma_gather_raw(
                                nc.gpsimd,
                                out_ap=msg[
                                    :, s0 * ROWF : (s0 + sw) * ROWF
                                ].rearrange("p (c f) -> p c f", f=ROWF),
                                in_ap=src,
                                idxs_ap=idxt[:],
                                num_idxs=n_idx,
                                elem_size=ROWF,
                                elem_step=ROWF,
                                queue_num=qn,
                            )
                            qn = (qn + 1) % NUM_QUEUES
                        nc.vector.tensor_tensor(
                            out=msg[:].rearrange("p (d f) -> p d f", f=BATCH),
                            in0=msg[:].rearrange("p (d f) -> p d f", f=BATCH),
                            in1=w_s[
                                :, cb * PACK : (cb + ccols) * PACK
                            ].to_broadcast([P, ccols * PACK, BATCH]),
                            op=mybir.AluOpType.mult,
                        )
                        for t in range(t0, t1):
                            d_t = int(prep.dt[t])
                            o = (int(prep.off[t]) - cb) * ROWF
                            base2 = msg[:, o : o + BATCH]
                            in_ap = bass.AP(
                                base2.tensor,
                                base2.offset,
                                [
                                    base2.ap[0],
                                    [1, BATCH],
                                    [BATCH, d_t * PACK],
                                ],
                            )
                            nc.vector.tensor_reduce(
                                out=ylayer[:, t * BATCH : (t + 1) * BATCH],
                                in_=in_ap,
                                axis=mybir.AxisListType.X,
                                op=mybir.AluOpType.add,
                            )
                    # write this half's slice and (if not last layer) AllGather
                    tcol0 = 0 if h == 0 else ts * BATCH
                    tcol1 = ts * BATCH if h == 0 else prep.tiles * BATCH
                    r0 = 0 if h == 0 else nps
                    r1 = nps if h == 0 else npc
                    nc.sync.dma_start(
                        out=dst[r0:r1, :].rearrange("(t p) f -> p t f", p=P),
                        in_=ylayer[:, tcol0:tcol1].rearrange(
                            "p (t f) -> p t f", f=BATCH
                        ),
                    )
                    if l < layers - 1:
                        nc.gpsimd.collective_compute(
                            "AllGather",
                            mybir.AluOpType.bypass,
                            replica_groups=[list(range(ncores))],
                            ins=[dst[r0:r1, :]],
                            outs=[ags[l][h][:, :]],
                        )
                        nc.sync.dma_start(
                            out=ybufs[l][
                                h * (npad // 2) : (h + 1) * (npad // 2), :
                            ],
                            in_=ags[l][h][:, :],
                        )

                emit_half(0)
                emit_half(1)
    nc.compile()
    return nc


def run(prep, trace=False):
    from concourse.bass_utils import run_bass_kernel_spmd

    nc = build_program(prep)
    in_maps = [
        {"xfull": prep.xpad, "idx": prep.idx16_all[k], "w": prep.w8[k]}
        for k in range(prep.ncores)
    ]
    res = run_bass_kernel_spmd(
        nc, in_maps, core_ids=list(range(prep.ncores)), trace=trace
    )
    y_concat = np.concatenate(
        [res.results[k]["yout"] for k in range(prep.ncores)], axis=0
    )
    return y_concat[prep.unshard], res


def kernel(x, weights, row, col):
    prep = _Prep(x, weights, row, col, N_NODES, NCORES, LAYERS)
    y, _ = run(prep, trace=False)
    return y


# revision 6
# speedup vs baseline: 1.0574x; 1.0574x over previous
"""Trainium2 Bass kernel for repeated sparse COO SpMM (GNN message passing).

y <- A @ y applied LAYERS times, A[row[e], col[e]] = weights[e].
N=100000 nodes, E=3200000 edges, B=16 features, 4 layers, 8 NeuronCores.

v6 strategy (1D partition by destination row):
  * Host: relabel nodes into 16 degree-snaked shards (core c owns shards
    c and c+8); per destination-tile (128 dests) the on-chip segment-sum
    is one strided DVE reduce with D = max degree in tile (degree-sorted
    tiles make the padding ~1%).
  * y is f32, packed 4 nodes per 256B row (dma_gather's stride quantum),
    so the whole graph fits ONE int16 gather window (25088 blocks). Each
    edge gathers the 256B block holding its source node; a host-built
    weight mask selects the node during the DVE multiply, and the
    strided reduce sums over D*4 sub-slots in f32.
  * dma_gather descriptor generation runs on one Q7 core pair per SWDGE
    queue; calls round-robin over all 4 queues so all 8 Q7 cores
    generate descriptors in parallel. The whole int16 index table is
    SBUF-resident (loaded once), so gathers never wait on index DMAs.
  * Each layer's AllGather is split in two (shards 0-7, then 8-15): the
    first fires mid-layer and overlaps the second half's gathers. The
    AllGather output IS the packed layout (same bytes), so the next
    layer gathers straight from it.
"""

import numpy as np

# ---------------------------------------------------------------- problem dims
N_NODES = 100000
N_EDGES = 3200000
BATCH = 16
LAYERS = 4
NCORES = 8
NSHARDS = 16
P = 128
PACK = 4  # nodes per 256B gather block (f32)
ROWF = PACK * BATCH  # 64 f32 per packed row

SLOT_BUDGET = 96  # msg-buffer slots per chunk (x256B per partition)
NUM_QUEUES = 4
CALL_SLOTS = 8  # slot-columns per dma_gather call (x128 = 1024 idxs)


def _mk_chunks(dt, t0, t1):
    chunks = []
    t = t0
    while t < t1:
        te = t
        acc = 0
        while te < t1 and (te == t or acc + dt[te] <= SLOT_BUDGET):
            acc += dt[te]
            te += 1
        chunks.append((t, te))
        t = te
    return chunks


class _Prep:
    """Host-side graph preprocessing, shared by kernel() and tests."""

    def __init__(self, x, weights, row, col, n_nodes, ncores, layers):
        n = n_nodes
        nps_real = n // NSHARDS  # real nodes per shard
        tiles_s = (nps_real + P - 1) // P  # tiles per shard
        nps = tiles_s * P  # padded shard size
        tiles = 2 * tiles_s  # tiles per core
        npc = 2 * nps  # positions per core
        npad = NSHARDS * nps
        nblocks = npad // PACK
        assert nblocks < 32768  # one int16 gather window

        row = np.asarray(row).astype(np.int64)
        col = np.asarray(col).astype(np.int64)
        weights = np.asarray(weights, dtype=np.float32)
        deg = np.bincount(row, minlength=n)

        # ascending-degree order, snake-assigned to 16 shards
        order = np.argsort(deg, kind="stable")
        blocks = order.reshape(nps_real, NSHARDS).copy()
        blocks[1::2] = blocks[1::2, ::-1]
        perm = np.empty(n, dtype=np.int64)
        for s in range(NSHARDS):
            perm[blocks[:, s]] = s * nps + np.arange(nps_real)

        new_row = perm[row]
        new_col = perm[col]

        # global position -> (core, local dest index)
        def to_core_local(p):
            v = p // nps
            return v % ncores, (v // ncores) * nps + p % nps

        # --- per-edge slot within its destination bucket -----------------
        eorder = np.argsort(new_row, kind="stable")
        sr = new_row[eorder]
        sc = new_col[eorder]
        sw_weights = weights[eorder]
        change = np.flatnonzero(np.diff(sr)) + 1
        starts = np.concatenate(([0], change))
        counts = np.diff(np.concatenate((starts, [len(sr)])))
        dests = sr[starts]
        j = np.arange(len(sr)) - np.repeat(starts, counts)  # rank in bucket

        dest_core, dest_local = to_core_local(dests)
        dest_ltile = dest_local // P
        e_core = np.repeat(dest_core, counts)
        e_ltile = np.repeat(dest_ltile, counts)
        e_p = np.repeat(dest_local, counts) % P

        dtc = np.ones((ncores, tiles), dtype=np.int64)
        np.maximum.at(dtc, (dest_core, dest_ltile), counts)
        # all cores share one program => tile widths must match across cores
        dt = dtc.max(axis=0)  # [tiles]
        off = np.concatenate(([0], np.cumsum(dt)[:-1]))  # col base per tile
        total_cols = int(dt.sum())

        # chunks per half (never straddle the shard boundary tile)
        chunks_a = _mk_chunks(dt, 0, tiles_s)
        chunks_b = _mk_chunks(dt, tiles_s, tiles)
        chunks = chunks_a + chunks_b
        chunk_cols = np.array([int(dt[a:b].sum()) for a, b in chunks])
        chunk_col_base = np.concatenate(([0], np.cumsum(chunk_cols)[:-1]))

        # --- per-edge slot column, weight mask, idx table ----------------
        e_slot = off[e_ltile] + j  # global slot column
        blk = sc // PACK
        sub = sc % PACK

        w8 = np.zeros((ncores, P, total_cols * PACK), dtype=np.float32)
        w8[e_core, e_p, e_slot * PACK + sub] = sw_weights

        # gather entry j -> partition j%128, free j//128 (slot col)
        e_entry = e_slot * P + e_p
        total_entries = total_cols * P
        flat_idx = np.zeros((ncores, total_entries), dtype=np.int16)
        flat_idx[e_core, e_entry] = blk.astype(np.int16)
        assert total_entries % 16 == 0
        wrapped = flat_idx.reshape(ncores, total_entries // 16, 16).transpose(
            0, 2, 1
        )
        idx16_all = np.ascontiguousarray(np.tile(wrapped, (1, 8, 1)))

        xp = np.zeros((npad, BATCH), dtype=np.float32)
        xp[perm] = np.asarray(x, dtype=np.float32)
        xpad = np.ascontiguousarray(xp.reshape(nblocks, ROWF))

        # unshard: node n -> y_concat[core*npc + local]
        pc, pl = to_core_local(perm)
        self.unshard = pc * npc + pl

        self.n_nodes = n
        self.ncores = ncores
        self.layers = layers
        self.tiles = tiles
        self.tiles_s = tiles_s
        self.nps = nps
        self.npc = npc
        self.npad = npad
        self.nblocks = nblocks
        self.dt = dt
        self.off = off
        self.chunks = chunks
        self.n_chunks_a = len(chunks_a)
        self.chunk_cols = chunk_cols
        self.chunk_col_base = chunk_col_base
        self.total_cols = total_cols
        self.total_entries = total_entries
        self.perm = perm
        self.w8 = w8  # f32 weight mask
        self.idx16_all = idx16_all
        self.xpad = xpad
        self.slots = int(total_cols) * P


_REG_CACHE = {}


def _dma_gather_raw(
    g, out_ap, in_ap, idxs_ap, num_idxs, elem_size, elem_step, queue_num
):
    """Non-transpose DRAM-source dma_gather without the (transpose-only)
    elem_size%256 restriction. Mirrors bass.BassGpSimd.dma_gather."""
    import concourse.mybir as mybir

    stride_bytes = elem_step * mybir.dt.size(in_ap.dtype)
    assert stride_bytes % 256 == 0
    _in_ap = g.lower_ap_dma(in_ap, for_custom_bir_dma=True)
    _idxs_ap = g.lower_ap(idxs_ap)
    _out_ap = g.lower_ap(out_ap)
    key = (id(g), num_idxs)
    if key not in _REG_CACHE:
        _REG_CACHE[key] = g.to_reg(num_idxs)
    return g.add_instruction(
        mybir.InstDMAGatherAnt(
            name=g.bass.get_next_instruction_name(),
            ins=[*_in_ap, _idxs_ap, g.lower_val_access(_REG_CACHE[key])],
            outs=[_out_ap],
            transpose=False,
            num_idxs=num_idxs,
            elem_size=elem_size,
            stride_bytes_256=stride_bytes // 256,
            gen_mode=0,
            single_packet=True,
            queue_num=queue_num,
            sbuf_tokens_per_rank=0,
            sbuf_free_dim_per_rank=0,
            sbuf_free_dim_pad_per_rank=0,
            sbuf_byte_offset=0,
        )
    )


def build_program(prep):
    import concourse.bass as bass
    import concourse.bacc as bacc
    import concourse.mybir as mybir
    import concourse.tile as tile

    ncores = prep.ncores
    npc = prep.npc
    nps = prep.nps
    npad = prep.npad
    layers = prep.layers
    ts = prep.tiles_s

    nc = bacc.Bacc(
        None,
        num_devices=ncores,
        num_swdge_queues=NUM_QUEUES,
    )
    f32 = mybir.dt.float32
    i16 = mybir.dt.int16
    xfull = nc.dram_tensor(
        "xfull", [prep.nblocks, ROWF], f32, kind="ExternalInput"
    )
    idx_d = nc.dram_tensor(
        "idx", [P, prep.total_entries // 16], i16, kind="ExternalInput"
    )
    w_d = nc.dram_tensor(
        "w", [P, prep.total_cols * PACK], f32, kind="ExternalInput"
    )
    yout = nc.dram_tensor("yout", [npc, BATCH], f32, kind="ExternalOutput")

    with tile.TileContext(nc) as tc:
        with (
            tc.tile_pool(name="res", bufs=1) as res_pool,
            tc.tile_pool(name="msgp", bufs=3) as msg_pool,
            tc.tile_pool(name="outp", bufs=1) as out_pool,
            tc.tile_pool(name="dram", bufs=1, space="DRAM") as dram_pool,
        ):
            w_s = res_pool.tile([P, prep.total_cols * PACK], f32, name="w_s")
            nc.sync.dma_start(out=w_s[:], in_=w_d[:])
            idx_s = res_pool.tile(
                [P, prep.total_entries // 16], i16, name="idx_s"
            )
            nc.sync.dma_start(out=idx_s[:], in_=idx_d[:])
            # prime the DVE dependency on the w_s load
            w_prime = res_pool.tile([P, 1], f32, name="w_prime")
            nc.vector.tensor_copy(out=w_prime[:], in_=w_s[:, 0:1])

            slices = [
                dram_pool.tile([npc, BATCH], f32, name=f"slice{i}")
                for i in range(max(layers - 1, 1))
            ]
            ags = [
                [
                    dram_pool.tile(
                        [npad // 2, BATCH],
                        f32,
                        addr_space="Shared",
                        name=f"ag{i}_{h}",
                    )
                    for h in range(2)
                ]
                for i in range(max(layers - 1, 1))
            ]
            ybufs = [
                dram_pool.tile([npad, BATCH], f32, name=f"ybuf{i}")
                for i in range(max(layers - 1, 1))
            ]

            qn = 0
            for l in range(layers):
                if l == 0:
                    src = xfull[:, :]
                else:
                    src = ybufs[l - 1][:, :].rearrange(
                        "(n k) b -> n (k b)", k=PACK
                    )
                dst = yout if l == layers - 1 else slices[l]
                ylayer = out_pool.tile(
                    [P, prep.tiles * BATCH], f32, name=f"ylayer{l}", tag="yl"
                )

                def emit_half(h):
                    nonlocal qn
                    lo = 0 if h == 0 else prep.n_chunks_a
                    hi = prep.n_chunks_a if h == 0 else len(prep.chunks)
                    for ci in range(lo, hi):
                        t0, t1 = prep.chunks[ci]
                        ccols = int(prep.chunk_cols[ci])
                        cb = int(prep.chunk_col_base[ci])
                        msg = msg_pool.tile(
                            [P, ccols * ROWF], f32, name="msg", tag="msg"
                        )
                        for s0 in range(0, ccols, CALL_SLOTS):
                            sw = min(CALL_SLOTS, ccols - s0)
                            n_idx = P * sw
                            eb = (cb + s0) * P
                            _dma_gather_raw(
                                nc.gpsimd,
                                out_ap=msg[
                                    :, s0 * ROWF : (s0 + sw) * ROWF
                                ].rearrange("p (c f) -> p c f", f=ROWF),
                                in_ap=src,
                                idxs_ap=idx_s[:, eb // 16 : (eb + n_idx) // 16],
                                num_idxs=n_idx,
                                elem_size=ROWF,
                                elem_step=ROWF,
                                queue_num=qn,
                            )
                            qn = (qn + 1) % NUM_QUEUES
                        nc.vector.tensor_tensor(
                            out=msg[:].rearrange("p (d f) -> p d f", f=BATCH),
                            in0=msg[:].rearrange("p (d f) -> p d f", f=BATCH),
                            in1=w_s[
                                :, cb * PACK : (cb + ccols) * PACK
                            ].to_broadcast([P, ccols * PACK, BATCH]),
                            op=mybir.AluOpType.mult,
                        )
                        for t in range(t0, t1):
                            d_t = int(prep.dt[t])
                            o = (int(prep.off[t]) - cb) * ROWF
                            base2 = msg[:, o : o + BATCH]
                            in_ap = bass.AP(
                                base2.tensor,
                                base2.offset,
                                [
                                    base2.ap[0],
                                    [1, BATCH],
                                    [BATCH, d_t * PACK],
                                ],
                            )
                            nc.vector.tensor_reduce(
                                out=ylayer[:, t * BATCH : (t + 1) * BATCH],
                                in_=in_ap,
                                axis=mybir.AxisListType.X,
                                op=mybir.AluOpType.add,
                            )
                    # write this half's slice and (if not last layer) AllGather
                    tcol0 = 0 if h == 0 else ts * BATCH
                    tcol1 = ts * BATCH if h == 0 else prep.tiles * BATCH
                    r0 = 0 if h == 0 else nps
                    r1 = nps if h == 0 else npc
                    nc.sync.dma_start(
                        out=dst[r0:r1, :].rearrange("(t p) f -> p t f", p=P),
                        in_=ylayer[:, tcol0:tcol1].rearrange(
                            "p (t f) -> p t f", f=BATCH
                        ),
                    )
                    if l < layers - 1:
                        nc.gpsimd.collective_compute(
                            "AllGather",
                            mybir.AluOpType.bypass,
                            replica_groups=[list(range(ncores))],
                            ins=[dst[r0:r1, :]],
                            outs=[ags[l][h][:, :]],
                        )
                        nc.sync.dma_start(
                            out=ybufs[l][
                                h * (npad // 2) : (h + 1) * (npad // 2), :
                            ],
                            in_=ags[l][h][:, :],
                        )

                emit_half(0)
                emit_half(1)
    nc.compile()
    return nc


def run(prep, trace=False):
    from concourse.bass_utils import run_bass_kernel_spmd

    nc = build_program(prep)
    in_maps = [
        {"xfull": prep.xpad, "idx": prep.idx16_all[k], "w": prep.w8[k]}
        for k in range(prep.ncores)
    ]
    res = run_bass_kernel_spmd(
        nc, in_maps, core_ids=list(range(prep.ncores)), trace=trace
    )
    y_concat = np.concatenate(
        [res.results[k]["yout"] for k in range(prep.ncores)], axis=0
    )
    return y_concat[prep.unshard], res


def kernel(x, weights, row, col):
    prep = _Prep(x, weights, row, col, N_NODES, NCORES, LAYERS)
    y, _ = run(prep, trace=False)
    return y


# revision 7
# speedup vs baseline: 1.0662x; 1.0084x over previous
"""Trainium2 Bass kernel for repeated sparse COO SpMM (GNN message passing).

y <- A @ y applied LAYERS times, A[row[e], col[e]] = weights[e].
N=100000 nodes, E=3200000 edges, B=16 features, 4 layers, 8 NeuronCores.

v9 strategy (1D partition by destination row):
  * Host: relabel nodes into 16 degree-snaked shards (core c owns shards
    c and c+8); per destination-tile (128 dests) the on-chip segment-sum
    is one strided DVE reduce with D = max degree in tile (degree-sorted
    tiles make the padding ~1%).
  * y is f32, packed 4 nodes per 256B row (dma_gather's stride quantum),
    so the whole graph fits ONE int16 gather window (25088 blocks). Each
    edge gathers the 256B block holding its source node; a host-built
    weight mask selects the node during the DVE multiply, and the
    strided reduce sums over D*4 sub-slots in f32.
  * dma_gather descriptor generation runs on one Q7 core pair per SWDGE
    queue; calls round-robin over all 4 queues so all 8 Q7 cores
    generate descriptors in parallel. The whole int16 index table is
    SBUF-resident (loaded once), so gathers never wait on index DMAs.
  * Each layer's AllGather is split in two (shards 0-7, then 8-15): the
    first fires mid-layer and overlaps the second half's gathers. The
    AllGather output IS the packed layout (same bytes), so the next
    layer gathers straight from it.
"""

import numpy as np

# ---------------------------------------------------------------- problem dims
N_NODES = 100000
N_EDGES = 3200000
BATCH = 16
LAYERS = 4
NCORES = 8
NSHARDS = 16
P = 128
PACK = 4  # nodes per 256B gather block (f32)
ROWF = PACK * BATCH  # 64 f32 per packed row

SLOT_BUDGET = 64  # msg-buffer slots per chunk (x256B per partition)
NUM_QUEUES = 4
CALL_SLOTS = 8  # slot-columns per dma_gather call (x128 = 1024 idxs)


def _mk_chunks(dt, t0, t1):
    chunks = []
    t = t0
    while t < t1:
        te = t
        acc = 0
        while te < t1 and (te == t or acc + dt[te] <= SLOT_BUDGET):
            acc += dt[te]
            te += 1
        chunks.append((t, te))
        t = te
    return chunks


class _Prep:
    """Host-side graph preprocessing, shared by kernel() and tests."""

    def __init__(self, x, weights, row, col, n_nodes, ncores, layers):
        n = n_nodes
        nps_real = n // NSHARDS  # real nodes per shard
        tiles_s = (nps_real + P - 1) // P  # tiles per shard
        nps = tiles_s * P  # padded shard size
        tiles = 2 * tiles_s  # tiles per core
        npc = 2 * nps  # positions per core
        npad = NSHARDS * nps
        nblocks = npad // PACK
        assert nblocks < 32768  # one int16 gather window

        row = np.asarray(row).astype(np.int64)
        col = np.asarray(col).astype(np.int64)
        weights = np.asarray(weights, dtype=np.float32)
        deg = np.bincount(row, minlength=n)

        # ascending-degree order, snake-assigned to 16 shards
        order = np.argsort(deg, kind="stable")
        blocks = order.reshape(nps_real, NSHARDS).copy()
        blocks[1::2] = blocks[1::2, ::-1]
        perm = np.empty(n, dtype=np.int64)
        for s in range(NSHARDS):
            perm[blocks[:, s]] = s * nps + np.arange(nps_real)

        new_row = perm[row]
        new_col = perm[col]

        # global position -> (core, local dest index)
        def to_core_local(p):
            v = p // nps
            return v % ncores, (v // ncores) * nps + p % nps

        # --- per-edge slot within its destination bucket -----------------
        eorder = np.argsort(new_row, kind="stable")
        sr = new_row[eorder]
        sc = new_col[eorder]
        sw_weights = weights[eorder]
        change = np.flatnonzero(np.diff(sr)) + 1
        starts = np.concatenate(([0], change))
        counts = np.diff(np.concatenate((starts, [len(sr)])))
        dests = sr[starts]
        j = np.arange(len(sr)) - np.repeat(starts, counts)  # rank in bucket

        dest_core, dest_local = to_core_local(dests)
        dest_ltile = dest_local // P
        e_core = np.repeat(dest_core, counts)
        e_ltile = np.repeat(dest_ltile, counts)
        e_p = np.repeat(dest_local, counts) % P

        dtc = np.ones((ncores, tiles), dtype=np.int64)
        np.maximum.at(dtc, (dest_core, dest_ltile), counts)
        # all cores share one program => tile widths must match across cores
        dt = dtc.max(axis=0)  # [tiles]
        off = np.concatenate(([0], np.cumsum(dt)[:-1]))  # col base per tile
        total_cols = int(dt.sum())

        # chunks per half (never straddle the shard boundary tile)
        chunks_a = _mk_chunks(dt, 0, tiles_s)
        chunks_b = _mk_chunks(dt, tiles_s, tiles)
        chunks = chunks_a + chunks_b
        chunk_cols = np.array([int(dt[a:b].sum()) for a, b in chunks])
        chunk_col_base = np.concatenate(([0], np.cumsum(chunk_cols)[:-1]))

        # --- per-edge slot column, weight mask, idx table ----------------
        e_slot = off[e_ltile] + j  # global slot column
        blk = sc // PACK
        sub = sc % PACK

        w8 = np.zeros((ncores, P, total_cols * PACK), dtype=np.float32)
        w8[e_core, e_p, e_slot * PACK + sub] = sw_weights

        # gather entry j -> partition j%128, free j//128 (slot col)
        e_entry = e_slot * P + e_p
        total_entries = total_cols * P
        flat_idx = np.zeros((ncores, total_entries), dtype=np.int16)
        flat_idx[e_core, e_entry] = blk.astype(np.int16)
        assert total_entries % 16 == 0
        wrapped = flat_idx.reshape(ncores, total_entries // 16, 16).transpose(
            0, 2, 1
        )
        idx16_all = np.ascontiguousarray(np.tile(wrapped, (1, 8, 1)))

        xp = np.zeros((npad, BATCH), dtype=np.float32)
        xp[perm] = np.asarray(x, dtype=np.float32)
        xpad = np.ascontiguousarray(xp.reshape(nblocks, ROWF))

        # unshard: node n -> y_concat[core*npc + local]
        pc, pl = to_core_local(perm)
        self.unshard = pc * npc + pl

        self.n_nodes = n
        self.ncores = ncores
        self.layers = layers
        self.tiles = tiles
        self.tiles_s = tiles_s
        self.nps = nps
        self.npc = npc
        self.npad = npad
        self.nblocks = nblocks
        self.dt = dt
        self.off = off
        self.chunks = chunks
        self.n_chunks_a = len(chunks_a)
        self.chunk_cols = chunk_cols
        self.chunk_col_base = chunk_col_base
        self.total_cols = total_cols
        self.total_entries = total_entries
        self.perm = perm
        self.w8 = w8  # f32 weight mask
        self.idx16_all = idx16_all
        self.xpad = xpad
        self.slots = int(total_cols) * P


_REG_CACHE = {}


def _dma_gather_raw(
    g, out_ap, in_ap, idxs_ap, num_idxs, elem_size, elem_step, queue_num
):
    """Non-transpose DRAM-source dma_gather without the (transpose-only)
    elem_size%256 restriction. Mirrors bass.BassGpSimd.dma_gather."""
    import concourse.mybir as mybir

    stride_bytes = elem_step * mybir.dt.size(in_ap.dtype)
    assert stride_bytes % 256 == 0
    _in_ap = g.lower_ap_dma(in_ap, for_custom_bir_dma=True)
    _idxs_ap = g.lower_ap(idxs_ap)
    _out_ap = g.lower_ap(out_ap)
    key = (id(g), num_idxs)
    if key not in _REG_CACHE:
        _REG_CACHE[key] = g.to_reg(num_idxs)
    return g.add_instruction(
        mybir.InstDMAGatherAnt(
            name=g.bass.get_next_instruction_name(),
            ins=[*_in_ap, _idxs_ap, g.lower_val_access(_REG_CACHE[key])],
            outs=[_out_ap],
            transpose=False,
            num_idxs=num_idxs,
            elem_size=elem_size,
            stride_bytes_256=stride_bytes // 256,
            gen_mode=0,
            single_packet=True,
            queue_num=queue_num,
            sbuf_tokens_per_rank=0,
            sbuf_free_dim_per_rank=0,
            sbuf_free_dim_pad_per_rank=0,
            sbuf_byte_offset=0,
        )
    )


def build_program(prep):
    import concourse.bass as bass
    import concourse.bacc as bacc
    import concourse.mybir as mybir
    import concourse.tile as tile

    ncores = prep.ncores
    npc = prep.npc
    nps = prep.nps
    npad = prep.npad
    layers = prep.layers
    ts = prep.tiles_s

    nc = bacc.Bacc(
        None,
        num_devices=ncores,
        num_swdge_queues=NUM_QUEUES,
    )
    f32 = mybir.dt.float32
    i16 = mybir.dt.int16
    xfull = nc.dram_tensor(
        "xfull", [prep.nblocks, ROWF], f32, kind="ExternalInput"
    )
    idx_d = nc.dram_tensor(
        "idx", [P, prep.total_entries // 16], i16, kind="ExternalInput"
    )
    w_d = nc.dram_tensor(
        "w", [P, prep.total_cols * PACK], f32, kind="ExternalInput"
    )
    yout = nc.dram_tensor("yout", [npc, BATCH], f32, kind="ExternalOutput")

    with tile.TileContext(nc) as tc:
        with (
            tc.tile_pool(name="res", bufs=1) as res_pool,
            tc.tile_pool(name="msgp", bufs=4) as msg_pool,
            tc.tile_pool(name="outp", bufs=1) as out_pool,
            tc.tile_pool(name="dram", bufs=1, space="DRAM") as dram_pool,
        ):
            w_s = res_pool.tile([P, prep.total_cols * PACK], f32, name="w_s")
            nc.sync.dma_start(out=w_s[:], in_=w_d[:])
            idx_s = res_pool.tile(
                [P, prep.total_entries // 16], i16, name="idx_s"
            )
            nc.sync.dma_start(out=idx_s[:], in_=idx_d[:])
            # prime the DVE dependency on the w_s load
            w_prime = res_pool.tile([P, 1], f32, name="w_prime")
            nc.vector.tensor_copy(out=w_prime[:], in_=w_s[:, 0:1])

            slices = [
                dram_pool.tile([npc, BATCH], f32, name=f"slice{i}")
                for i in range(max(layers - 1, 1))
            ]
            ags = [
                [
                    dram_pool.tile(
                        [npad // 2, BATCH],
                        f32,
                        addr_space="Shared",
                        name=f"ag{i}_{h}",
                    )
                    for h in range(2)
                ]
                for i in range(max(layers - 1, 1))
            ]
            ybufs = [
                dram_pool.tile([npad, BATCH], f32, name=f"ybuf{i}")
                for i in range(max(layers - 1, 1))
            ]

            qn = 0
            for l in range(layers):
                if l == 0:
                    src = xfull[:, :]
                else:
                    src = ybufs[l - 1][:, :].rearrange(
                        "(n k) b -> n (k b)", k=PACK
                    )
                dst = yout if l == layers - 1 else slices[l]
                ylayer = out_pool.tile(
                    [P, prep.tiles * BATCH], f32, name=f"ylayer{l}", tag="yl"
                )

                def emit_half(h):
                    nonlocal qn
                    lo = 0 if h == 0 else prep.n_chunks_a
                    hi = prep.n_chunks_a if h == 0 else len(prep.chunks)
                    for ci in range(lo, hi):
                        t0, t1 = prep.chunks[ci]
                        ccols = int(prep.chunk_cols[ci])
                        cb = int(prep.chunk_col_base[ci])
                        msg = msg_pool.tile(
                            [P, ccols * ROWF], f32, name="msg", tag="msg"
                        )
                        for s0 in range(0, ccols, CALL_SLOTS):
                            sw = min(CALL_SLOTS, ccols - s0)
                            n_idx = P * sw
                            eb = (cb + s0) * P
                            _dma_gather_raw(
                                nc.gpsimd,
                                out_ap=msg[
                                    :, s0 * ROWF : (s0 + sw) * ROWF
                                ].rearrange("p (c f) -> p c f", f=ROWF),
                                in_ap=src,
                                idxs_ap=idx_s[:, eb // 16 : (eb + n_idx) // 16],
                                num_idxs=n_idx,
                                elem_size=ROWF,
                                elem_step=ROWF,
                                queue_num=qn,
                            )
                            qn = (qn + 1) % NUM_QUEUES
                        nc.vector.tensor_tensor(
                            out=msg[:].rearrange("p (d f) -> p d f", f=BATCH),
                            in0=msg[:].rearrange("p (d f) -> p d f", f=BATCH),
                            in1=w_s[
                                :, cb * PACK : (cb + ccols) * PACK
                            ].to_broadcast([P, ccols * PACK, BATCH]),
                            op=mybir.AluOpType.mult,
                        )
                        for t in range(t0, t1):
                            d_t = int(prep.dt[t])
                            o = (int(prep.off[t]) - cb) * ROWF
                            base2 = msg[:, o : o + BATCH]
                            in_ap = bass.AP(
                                base2.tensor,
                                base2.offset,
                                [
                                    base2.ap[0],
                                    [1, BATCH],
                                    [BATCH, d_t * PACK],
                                ],
                            )
                            nc.vector.tensor_reduce(
                                out=ylayer[:, t * BATCH : (t + 1) * BATCH],
                                in_=in_ap,
                                axis=mybir.AxisListType.X,
                                op=mybir.AluOpType.add,
                            )
                    # write this half's slice and (if not last layer) AllGather
                    tcol0 = 0 if h == 0 else ts * BATCH
                    tcol1 = ts * BATCH if h == 0 else prep.tiles * BATCH
                    r0 = 0 if h == 0 else nps
                    r1 = nps if h == 0 else npc
                    nc.sync.dma_start(
                        out=dst[r0:r1, :].rearrange("(t p) f -> p t f", p=P),
                        in_=ylayer[:, tcol0:tcol1].rearrange(
                            "p (t f) -> p t f", f=BATCH
                        ),
                    )
                    if l < layers - 1:
                        nc.gpsimd.collective_compute(
                            "AllGather",
                            mybir.AluOpType.bypass,
                            replica_groups=[list(range(ncores))],
                            ins=[dst[r0:r1, :]],
                            outs=[ags[l][h][:, :]],
                        )
                        nc.sync.dma_start(
                            out=ybufs[l][
                                h * (npad // 2) : (h + 1) * (npad // 2), :
                            ],
                            in_=ags[l][h][:, :],
                        )

                emit_half(0)
                emit_half(1)
    nc.compile()
    return nc


def run(prep, trace=False):
    from concourse.bass_utils import run_bass_kernel_spmd

    nc = build_program(prep)
    in_maps = [
        {"xfull": prep.xpad, "idx": prep.idx16_all[k], "w": prep.w8[k]}
        for k in range(prep.ncores)
    ]
    res = run_bass_kernel_spmd(
        nc, in_maps, core_ids=list(range(prep.ncores)), trace=trace
    )
    y_concat = np.concatenate(
        [res.results[k]["yout"] for k in range(prep.ncores)], axis=0
    )
    return y_concat[prep.unshard], res


def kernel(x, weights, row, col):
    prep = _Prep(x, weights, row, col, N_NODES, NCORES, LAYERS)
    y, _ = run(prep, trace=False)
    return y
